# revision 1
# baseline (speedup 1.0000x reference)
"""Paged GQA decode attention (B=64, HQ=32, HKV=8, D=128) on 8 TRN2 NeuronCores.

Strategy: data-parallel over requests with host-side load balancing.
 - Sort the 64 requests by context_lens descending; slot r of core c gets the
   rank-(r*8+c) request, so every core's slot-r request has a similar length.
 - Each slot is padded to the max-of-8 chunk count (chunks of 128 tokens), so
   all 8 cores execute the SAME static program (SPMD) on different data.
 - Host gathers each request's KV blocks (honoring block_tables) into per-core
   shards: K pre-transposed to [d, l] tiles (no on-chip transposes), V natural
   [l, d]. K is bf16; V fp8e4m3 (quantization errors largely cancel in the
   softmax ratio). Chunks stream in GRP-sized DMA groups that may span request
   slots (SWDGE path measured fastest at 8-core load).
 - Per chunk on device: scores_T[l,hq] = K_h^T.T @ qT (8 matmuls), then
   E = exp(scores + bias) on ScalarE where bias is 0 / -30 per token
   (masks padded/invalid tokens), then PV accumulation acc[hq,d] += E_h.T @ V_h
   (8 col-tiled matmuls into two PSUM banks) and a ones-matmul for the
   softmax denominator. Final division happens on host.
"""

import math
import os
import sys
from contextlib import ExitStack

import numpy as np
import ml_dtypes  # noqa: F401  (numpy bf16/fp8 dtypes)

for _p in ("/opt/trn_rl_repo", "/root/.axon_site/_ro/trn_rl_repo"):
    if os.path.isdir(_p) and _p not in sys.path:
        sys.path.insert(0, _p)
        break

import concourse.bass as bass  # noqa: F401
import concourse.tile as tile
from concourse import bacc, mybir
from concourse.bass_utils import run_bass_kernel_spmd

B, HQ, HKV, D, BS, MB = 64, 32, 8, 128, 16, 128
G = HQ // HKV              # 4 query heads per kv head
SCALE = 0.08838834764831845
NCORES = 8
SLOTS = B // NCORES        # 8 request slots per core
CHUNK = 128                # tokens per chunk (= SBUF partitions)
BPC = CHUNK // BS          # blocks per chunk = 8
ROW = HKV * D              # 1024 elements per token row
NEG = -30.0                # additive mask for invalid tokens
VSHIFT = -2.0              # shift valid scores so exp() fits fp8e4m3 range
GRP = 4                    # chunks per DMA group (groups may span slots)
KV_BUFS = 6                # group tiles in flight
K_ENG = "gpsimd"           # DMA issue engine for K: gpsimd|sync|scalar
V_ENG = "gpsimd"           # DMA issue engine for V
K_DT = "bf16"              # K/q dtype: "f32" | "bf16" | "fp8"
V_DT = "bf16"              # V/E dtype: "f32" | "bf16" | "fp8"

last_results = None        # stashed BassKernelResults for test.py

_prog_cache = {}


def _mdt(name):
    return {"f32": mybir.dt.float32, "bf16": mybir.dt.bfloat16,
            "fp8": mybir.dt.float8e4}[name]


def _ndt(name):
    return mybir.dt.np(_mdt(name))


def _build_program(s_counts, reps=1, dma_only=False):
    f32 = mybir.dt.float32
    kdt, vdt = _mdt(K_DT), _mdt(V_DT)
    C_total = sum(s_counts)
    NG = C_total // GRP
    nc = bacc.Bacc()

    k_d = nc.declare_dram_parameter("k", [NG, D, GRP * ROW], kdt,
                                    isOutput=False)
    v_d = nc.declare_dram_parameter("v", [NG, CHUNK, GRP * ROW], vdt,
                                    isOutput=False)
    qT_d = nc.declare_dram_parameter("qT", [D, SLOTS * HQ], kdt, isOutput=False)
    bias_d = nc.declare_dram_parameter("bias", [CHUNK, C_total], f32,
                                       isOutput=False)
    out_d = nc.declare_dram_parameter("out", [SLOTS, HKV, G, D], f32,
                                      isOutput=True)
    den_d = nc.declare_dram_parameter("den", [SLOTS, HQ], f32, isOutput=True)

    EXP = mybir.ActivationFunctionType.Exp

    with tile.TileContext(nc) as tc, ExitStack() as ctx:
        kpool = ctx.enter_context(tc.tile_pool(name="kp", bufs=KV_BUFS))
        vpool = ctx.enter_context(tc.tile_pool(name="vp", bufs=KV_BUFS))
        epool = ctx.enter_context(tc.tile_pool(name="e", bufs=3))
        const = ctx.enter_context(tc.tile_pool(name="cst", bufs=1))
        spsum = ctx.enter_context(tc.tile_pool(name="sp", bufs=2, space="PSUM"))
        apsum = ctx.enter_context(tc.tile_pool(name="ac", bufs=2, space="PSUM"))
        dpsum = ctx.enter_context(tc.tile_pool(name="dp", bufs=2, space="PSUM"))

        bias_t = const.tile([CHUNK, C_total], f32)
        nc.sync.dma_start(bias_t[:], bias_d[:])
        q_all = const.tile([D, SLOTS * HQ], kdt)
        nc.sync.dma_start(q_all[:], qT_d[:])
        # ones on ScalarE so the denominator matmul's deps stay in the single
        # ACT semaphore domain (PE matmuls support only one sync wait).
        ones = const.tile([CHUNK, 1], vdt)
        nc.scalar.activation(ones[:], bias_t[:, 0:1],
                             mybir.ActivationFunctionType.Identity,
                             bias=1.0, scale=0.0)
        # dummy matmul absorbs the q_all DMA wait so the first real matmul
        # only waits on its k/v DMA.
        dmy = spsum.tile([1, 1], f32, tag="sco")
        nc.tensor.matmul(dmy[:], q_all[0:1, 0:1], q_all[0:1, 0:1],
                         start=True, stop=True)

        def emit_body():
            cur = {}
            gc = 0
            for r in range(SLOTS):
                S_r = s_counts[r]
                qt = q_all[:, r * HQ:(r + 1) * HQ]
                acc_a = apsum.tile([CHUNK, D], f32, tag="acca")
                acc_b = apsum.tile([CHUNK, D], f32, tag="accb")
                den_p = dpsum.tile([HQ, 1], f32, tag="den")
                for j in range(S_r):
                    g, half = divmod(gc + j, GRP)
                    if half == 0 or "k" not in cur:
                        cur["k"] = kpool.tile([D, GRP * ROW], kdt,
                                              tag="kg", name="kg")
                        getattr(nc, K_ENG).dma_start(cur["k"][:], k_d[g])
                        cur["v"] = vpool.tile([CHUNK, GRP * ROW], vdt,
                                              tag="vg", name="vg")
                        getattr(nc, V_ENG).dma_start(cur["v"][:], v_d[g])
                    kt = cur["k"][:, half * ROW:(half + 1) * ROW]
                    vt = cur["v"][:, half * ROW:(half + 1) * ROW]
                    if dma_only:
                        continue

                    sco = spsum.tile([CHUNK, HQ], f32, tag="sco")
                    for h in range(HKV):
                        nc.tensor.matmul(
                            sco[:, h * G:(h + 1) * G],
                            kt[:, h * D:(h + 1) * D],
                            qt[:, h * G:(h + 1) * G],
                            start=True, stop=True,
                        )
                    et = epool.tile([CHUNK, HQ], vdt)
                    nc.scalar.activation(
                        et[:], sco[:], EXP,
                        bias=bias_t[:, gc + j:gc + j + 1], scale=1.0,
                    )
                    st, sp = (j == 0), (j == S_r - 1)
                    for h in range(HKV):
                        accp = acc_a if h < 4 else acc_b
                        jj = h % 4
                        nc.tensor.matmul(
                            accp[32 * jj:32 * jj + G, :],
                            et[:, h * G:(h + 1) * G],
                            vt[:, h * D:(h + 1) * D],
                            start=st, stop=sp,
                            tile_position=(0, 32 * jj),
                        )
                    nc.tensor.matmul(den_p[:], et[:], ones[:],
                                     start=st, stop=sp)
                out_sa = epool.tile([CHUNK, D], f32, tag="outa")
                out_sb = epool.tile([CHUNK, D], f32, tag="outb")
                den_s = epool.tile([HQ, 1], f32, tag="dens")
                if not dma_only:
                    nc.scalar.copy(out_sa[:], acc_a[:])
                    nc.scalar.copy(out_sb[:], acc_b[:])
                    nc.scalar.copy(den_s[:], den_p[:])
                else:
                    nc.vector.tensor_copy(out_sa[:], cur["k"][:, 0:D])
                    nc.vector.tensor_copy(out_sb[:], cur["v"][:, 0:D])
                    nc.vector.tensor_copy(den_s[:], bias_t[0:HQ, 0:1])
                for h in range(HKV):
                    srcp = out_sa if h < 4 else out_sb
                    jj = h % 4
                    nc.sync.dma_start(out_d[r, h], srcp[32 * jj:32 * jj + G, :])
                nc.sync.dma_start(den_d[r], den_s[:])
                gc += S_r

        if reps == 1:
            emit_body()
        else:
            with tc.For_i(0, reps, 1):
                emit_body()
    nc.compile()
    return nc


def _get_program(s_counts):
    if s_counts not in _prog_cache:
        _prog_cache[s_counts] = _build_program(s_counts)
    return _prog_cache[s_counts]


def _make_schedule(context_lens):
    L = context_lens.astype(np.int64)
    order = np.argsort(-L, kind="stable")
    s_counts = []
    for r in range(SLOTS):
        grp = order[r * NCORES:(r + 1) * NCORES]
        s_counts.append(max(1, math.ceil(int(L[grp].max()) / CHUNK)))
    rem = (-sum(s_counts)) % GRP
    s_counts[-1] += rem  # pad stream so DMA groups tile it exactly
    return order, tuple(s_counts)


def _build_in_maps(q, k_cache, v_cache, block_tables, L, order, s_counts):
    np_k, np_v = _ndt(K_DT), _ndt(V_DT)
    C_total = sum(s_counts)
    nblocks_total = k_cache.shape[0]
    kf = k_cache.reshape(nblocks_total, BS, ROW)
    vf = v_cache.reshape(nblocks_total, BS, ROW)

    in_maps = []
    core_reqs = []
    for c in range(NCORES):
        karr = np.empty((C_total, D, ROW), np_k)
        varr = np.empty((C_total, CHUNK, ROW), np_v)
        biasT = np.empty((C_total, CHUNK), np.float32)
        qT = np.empty((D, SLOTS * HQ), np_k)
        reqs = []
        gc = 0
        for r in range(SLOTS):
            b = int(order[r * NCORES + c])
            reqs.append(b)
            S_r = s_counts[r]
            blocks = np.clip(block_tables[b, :S_r * BPC].astype(np.int64),
                             0, nblocks_total - 1)
            kreq = kf[blocks].reshape(S_r, CHUNK, HKV, D)
            karr[gc:gc + S_r] = \
                kreq.transpose(0, 3, 2, 1).reshape(S_r, D, ROW)
            varr[gc:gc + S_r] = vf[blocks].reshape(S_r, CHUNK, ROW)
            tok = np.arange(S_r * CHUNK, dtype=np.int64)
            biasT[gc:gc + S_r] = np.where(tok < int(L[b]), VSHIFT, NEG) \
                .astype(np.float32).reshape(S_r, CHUNK)
            qT[:, r * HQ:(r + 1) * HQ] = (q[b] * SCALE).T
            gc += S_r
        # repack into GRP-chunk DMA groups: partition-major within a group
        kg = np.ascontiguousarray(
            karr.reshape(C_total // GRP, GRP, D, ROW).transpose(0, 2, 1, 3)
        ).reshape(C_total // GRP, D, GRP * ROW)
        vg = np.ascontiguousarray(
            varr.reshape(C_total // GRP, GRP, CHUNK, ROW).transpose(0, 2, 1, 3)
        ).reshape(C_total // GRP, CHUNK, GRP * ROW)
        in_maps.append({
            "k": kg, "v": vg, "qT": qT,
            "bias": np.ascontiguousarray(biasT.T),
        })
        core_reqs.append(reqs)
    return in_maps, core_reqs


def kernel(q, k_cache, v_cache, block_tables, context_lens):
    global last_results
    q = np.asarray(q, dtype=np.float32)
    k_cache = np.asarray(k_cache, dtype=np.float32)
    v_cache = np.asarray(v_cache, dtype=np.float32)
    block_tables = np.asarray(block_tables, dtype=np.int32)
    context_lens = np.asarray(context_lens, dtype=np.int32)

    L = context_lens.astype(np.int64)
    order, s_counts = _make_schedule(context_lens)
    nc = _get_program(s_counts)
    in_maps, core_reqs = _build_in_maps(
        q, k_cache, v_cache, block_tables, L, order, s_counts)

    res = run_bass_kernel_spmd(
        nc, in_maps, list(range(NCORES)),
        trace=bool(os.environ.get("KBASS_TRACE")),
    )
    last_results = res

    out = np.empty((B, HQ, D), np.float32)
    for c in range(NCORES):
        acc = res.results[c]["out"].reshape(SLOTS, HQ, D)
        den = np.maximum(res.results[c]["den"].reshape(SLOTS, HQ), 1e-30)
        o = acc / den[:, :, None]
        for r, b in enumerate(core_reqs[c]):
            out[b] = o[r]
    return out



# revision 2
# speedup vs baseline: 1.3316x; 1.3316x over previous
"""Paged GQA decode attention (B=64, HQ=32, HKV=8, D=128) on 8 TRN2 NeuronCores.

Strategy: flat chunk-parallel SPMD.
 - Every request is cut into 128-token chunks (533 total for this seed); the
   flat chunk list is split evenly across the 8 cores (padded to a DMA-group
   multiple), so all cores stream the same byte count and run one program.
 - No softmax-max pass: scores are shifted by a fixed VSHIFT and masked with
   an additive bias (0/-30), so partial (numerator, denominator) sums over
   disjoint token sets simply add — the host merges per-request partials.
 - Host gathers each chunk's KV blocks (honoring block_tables) into one
   contiguous stream per core: K pre-transposed to [d, token] tiles, V
   natural [token, d], both bf16, packed K|V into one [128, 8K] group tile
   per GRP=4 chunks = one 2 MB HWDGE DMA (nc.sync) per group.
 - Per chunk on device: scores[tok, hq] = K_h^T.T @ qT_c (8 matmuls into
   PSUM), E = exp(scores + bias_c) on ScalarE (bias masks invalid/padded
   tokens), then PV: acc[d, 4] = V_h.T @ E_h (8 matmuls, start/stop=True)
   into per-chunk columns of a PSUM bank that holds 16 chunks' partials,
   plus a ones-matmul denominator per chunk. Every 16 chunks the bank is
   copied to SBUF (DVE) and DMA'd out (gpsimd). Final division on host.
"""

import math
import os
import sys
from contextlib import ExitStack

import numpy as np
import ml_dtypes  # noqa: F401  (numpy bf16 dtype)

for _p in ("/opt/trn_rl_repo", "/root/.axon_site/_ro/trn_rl_repo"):
    if os.path.isdir(_p) and _p not in sys.path:
        sys.path.insert(0, _p)
        break

import concourse.bass as bass  # noqa: F401
import concourse.tile as tile
from concourse import bacc, mybir
from concourse.bass_utils import run_bass_kernel_spmd

B, HQ, HKV, D, BS, MB = 64, 32, 8, 128, 16, 128
G = HQ // HKV              # 4 query heads per kv head
SCALE = 0.08838834764831845
NCORES = 8
CHUNK = 128                # tokens per chunk (= SBUF partitions)
BPC = CHUNK // BS          # blocks per chunk = 8
ROW = HKV * D              # 1024 elements per token row
NEG = -30.0                # additive mask for invalid tokens
VSHIFT = -2.0              # fixed score shift (replaces softmax max pass)
GRP = 4                    # chunks per DMA group (one combined K|V transfer)
PGRP = 16                  # chunks per PSUM accumulation bank (32 cols each)
KV_BUFS = 5                # group tiles in flight (2 MB each)
KV_ENG = "sync"            # engine issuing the K|V group DMAs
OUT_ENG = "gpsimd"         # engine issuing staging/output DMAs

last_results = None        # stashed BassKernelResults for test.py

_prog_cache = {}

_bf16 = mybir.dt.bfloat16
_f32 = mybir.dt.float32
np_bf16 = mybir.dt.np(_bf16)


def _build_program(C):
    """C = chunks per core (multiple of GRP)."""
    NG = C // GRP
    NP = (C + PGRP - 1) // PGRP
    nc = bacc.Bacc()

    kv_d = nc.declare_dram_parameter("kv", [NG, CHUNK, 2 * GRP * ROW], _bf16,
                                     isOutput=False)
    qc_d = nc.declare_dram_parameter("qc", [D, C * HQ], _bf16, isOutput=False)
    bias_d = nc.declare_dram_parameter("bias", [CHUNK, C], _f32,
                                       isOutput=False)
    acc_d = nc.declare_dram_parameter("acc", [NP, D, PGRP * HQ], _f32,
                                      isOutput=True)
    den_d = nc.declare_dram_parameter("den", [HQ, C], _f32, isOutput=True)

    EXP = mybir.ActivationFunctionType.Exp

    with tile.TileContext(nc) as tc, ExitStack() as ctx:
        kvpool = ctx.enter_context(tc.tile_pool(name="kv", bufs=KV_BUFS))
        epool = ctx.enter_context(tc.tile_pool(name="e", bufs=3))
        stage = ctx.enter_context(tc.tile_pool(name="st", bufs=2))
        const = ctx.enter_context(tc.tile_pool(name="cst", bufs=1))
        spsum = ctx.enter_context(tc.tile_pool(name="sp", bufs=2, space="PSUM"))
        apsum = ctx.enter_context(tc.tile_pool(name="ac", bufs=2, space="PSUM"))
        dpsum = ctx.enter_context(tc.tile_pool(name="dp", bufs=2, space="PSUM"))

        bias_t = const.tile([CHUNK, C], _f32)
        getattr(nc, OUT_ENG).dma_start(bias_t[:], bias_d[:])
        q_all = const.tile([D, C * HQ], _bf16)
        getattr(nc, OUT_ENG).dma_start(q_all[:], qc_d[:])
        den_s = const.tile([HQ, C], _f32)
        # ones on ScalarE so the denominator matmul's deps stay in the single
        # ACT semaphore domain (PE matmuls support only one sync wait).
        ones = const.tile([CHUNK, 1], _bf16)
        nc.scalar.activation(ones[:], bias_t[:, 0:1],
                             mybir.ActivationFunctionType.Identity,
                             bias=1.0, scale=0.0)
        # dummy matmul absorbs the q DMA wait so the first real matmul
        # only waits on its kv DMA.
        dmy = spsum.tile([1, 1], _f32, tag="sco")
        nc.tensor.matmul(dmy[:], q_all[0:1, 0:1], q_all[0:1, 0:1],
                         start=True, stop=True)

        kvt = None
        accg = deng = None
        for c in range(C):
            g, half = divmod(c, GRP)
            if half == 0:
                kvt = kvpool.tile([CHUNK, 2 * GRP * ROW], _bf16, tag="kv",
                                  name="kv")
                getattr(nc, KV_ENG).dma_start(kvt[:], kv_d[g])
            kt = kvt[:, half * ROW:(half + 1) * ROW]
            vt = kvt[:, (GRP + half) * ROW:(GRP + half + 1) * ROW]

            sco = spsum.tile([CHUNK, HQ], _f32, tag="sco")
            for h in range(HKV):
                nc.tensor.matmul(
                    sco[:, h * G:(h + 1) * G],
                    kt[:, h * D:(h + 1) * D],
                    q_all[:, c * HQ + h * G:c * HQ + (h + 1) * G],
                    start=True, stop=True,
                )
            et = epool.tile([CHUNK, HQ], _bf16)
            nc.scalar.activation(et[:], sco[:], EXP,
                                 bias=bias_t[:, c:c + 1], scale=1.0)

            jm = c % PGRP
            if jm == 0:
                accg = apsum.tile([D, PGRP * HQ], _f32, tag="acc")
                deng = dpsum.tile([HQ, PGRP], _f32, tag="den")
            for h in range(HKV):
                nc.tensor.matmul(
                    accg[:, jm * HQ + h * G:jm * HQ + (h + 1) * G],
                    vt[:, h * D:(h + 1) * D],
                    et[:, h * G:(h + 1) * G],
                    start=True, stop=True,
                )
            nc.tensor.matmul(deng[:, jm:jm + 1], et[:], ones[:],
                             start=True, stop=True)

            if jm == PGRP - 1 or c == C - 1:
                p = c // PGRP
                used = (jm + 1) * HQ
                sacc = stage.tile([D, PGRP * HQ], _f32, tag="sacc")
                nc.vector.tensor_copy(sacc[:, :used], accg[:, :used])
                getattr(nc, OUT_ENG).dma_start(acc_d[p], sacc[:])
                nc.scalar.copy(den_s[:, p * PGRP:p * PGRP + jm + 1],
                               deng[:, :jm + 1])
        getattr(nc, OUT_ENG).dma_start(den_d[:], den_s[:])
    nc.compile()
    return nc


def _get_program(C):
    if C not in _prog_cache:
        _prog_cache[C] = _build_program(C)
    return _prog_cache[C]


def _make_schedule(context_lens):
    """Flat chunk list → per-core spans of C chunks each."""
    L = context_lens.astype(np.int64)
    chunks = []  # (request, chunk_idx)
    for b in range(B):
        for j in range(max(1, math.ceil(int(L[b]) / CHUNK))):
            chunks.append((b, j))
    C = math.ceil(len(chunks) / NCORES)
    C = math.ceil(C / GRP) * GRP
    pad = C * NCORES - len(chunks)
    chunks += [(-1, 0)] * pad
    return chunks, C


def _build_in_maps(q, k_cache, v_cache, block_tables, L, chunks, C):
    NG = C // GRP
    nblocks_total = k_cache.shape[0]
    kf = k_cache.reshape(nblocks_total, BS, ROW)
    vf = v_cache.reshape(nblocks_total, BS, ROW)
    qT = np.empty((B, D, HQ), np_bf16)
    for b in range(B):
        qT[b] = (q[b] * SCALE).T
    tok = np.arange(CHUNK, dtype=np.int64)

    in_maps = []
    for cidx in range(NCORES):
        span = chunks[cidx * C:(cidx + 1) * C]
        karr = np.zeros((C, D, ROW), np_bf16)
        varr = np.zeros((C, CHUNK, ROW), np_bf16)
        biasT = np.full((C, CHUNK), NEG, np.float32)
        qc = np.zeros((D, C * HQ), np_bf16)
        for i, (b, j) in enumerate(span):
            if b < 0:
                continue
            blocks = np.clip(
                block_tables[b, j * BPC:(j + 1) * BPC].astype(np.int64),
                0, nblocks_total - 1)
            kreq = kf[blocks].reshape(CHUNK, HKV, D)
            karr[i] = kreq.transpose(2, 1, 0).reshape(D, ROW)
            varr[i] = vf[blocks].reshape(CHUNK, ROW)
            biasT[i] = np.where(j * CHUNK + tok < int(L[b]), VSHIFT, NEG)
            qc[:, i * HQ:(i + 1) * HQ] = qT[b]
        kg = np.ascontiguousarray(
            karr.reshape(NG, GRP, D, ROW).transpose(0, 2, 1, 3)
        ).reshape(NG, D, GRP * ROW)
        vg = np.ascontiguousarray(
            varr.reshape(NG, GRP, CHUNK, ROW).transpose(0, 2, 1, 3)
        ).reshape(NG, CHUNK, GRP * ROW)
        in_maps.append({
            "kv": np.concatenate([kg, vg], axis=2),
            "qc": qc,
            "bias": np.ascontiguousarray(biasT.T),
        })
    return in_maps


def kernel(q, k_cache, v_cache, block_tables, context_lens):
    global last_results
    q = np.asarray(q, dtype=np.float32)
    k_cache = np.asarray(k_cache, dtype=np.float32)
    v_cache = np.asarray(v_cache, dtype=np.float32)
    block_tables = np.asarray(block_tables, dtype=np.int32)
    context_lens = np.asarray(context_lens, dtype=np.int32)

    L = context_lens.astype(np.int64)
    chunks, C = _make_schedule(context_lens)
    nc = _get_program(C)
    in_maps = _build_in_maps(q, k_cache, v_cache, block_tables, L, chunks, C)

    res = run_bass_kernel_spmd(
        nc, in_maps, list(range(NCORES)),
        trace=bool(os.environ.get("KBASS_TRACE")),
    )
    last_results = res

    NP = (C + PGRP - 1) // PGRP
    acc = np.zeros((B, HQ, D), np.float64)
    den = np.zeros((B, HQ), np.float64)
    for cidx in range(NCORES):
        # [NP, D, PGRP*HQ] -> per-chunk [HQ, D]
        pacc = res.results[cidx]["acc"].reshape(NP, D, PGRP, HQ) \
            .transpose(0, 2, 3, 1).reshape(NP * PGRP, HQ, D)
        pden = res.results[cidx]["den"]  # [HQ, C]
        for i, (b, j) in enumerate(chunks[cidx * C:(cidx + 1) * C]):
            if b < 0:
                continue
            acc[b] += pacc[i]
            den[b] += pden[:, i]
    out = acc / np.maximum(den, 1e-30)[:, :, None]
    return out.astype(np.float32)
